# revision 17
# baseline (speedup 1.0000x reference)
"""GroupedQueryAttention (B=2, N=2048, D=2048, H=16, HKV=4, HD=128) on 8 trn2 cores.

Sharding: core c handles (batch b = c//4, kv-head g = c%4): 4 q-heads + 1 kv head.
RoPE (with the reference's sin==cos quirk) is folded into Wq/Wk host-side:
  q_rot = cos ⊙ (M q) with constant M = [[I,-I],[I,I]] acting on head-dim halves,
so on-device RoPE is just an elementwise multiply by a precomputed cos table.
The softmax scale is folded into Wq. All matmuls run in bf16 with fp32 PSUM
accumulation. Attention uses a transpose-free layout chain:
  qT,kT [hd,n]; ST = kT.T @ qT [m,n]; exp on ScalarE; OT += v.T@ST (v as [m,hd]);
  softmax denominators via an all-ones stationary matmul; normalize with DVE
  fast-reciprocal. Per-head chunked AllGathers (bf16) overlap with attention of
  later heads; the output projection accumulates per gathered slab into an SBUF
  fp32 accumulator in transposed [d, n] layout (Wo chunks stationary), and the
  host transposes each core's [512, 2048] slice back.
Host gathers: out[b][:, g*512:(g+1)*512] = core (b,g) output transposed.
"""

import sys
import types

import numpy as np

B, N, D = 2, 2048, 2048
H, HKV, HD = 16, 4, 128
G = H // HKV  # q heads per kv head = 4
N_CORES = 8
ROPE_BASE = 10000.0
DSLICE = D // G  # 512 output columns per core
JL = G * HD  # 512 local attention-output rows per core


def _install_axon_ntff_hook():
    """This container's antenv lacks axon_hooks; inject it so trace=True works."""
    if "antenv.axon_hooks" in sys.modules:
        return
    try:
        from trn_agent_boot.trn_boot import _ntff_profile_via_ctypes

        hook = _ntff_profile_via_ctypes("/opt/axon/libaxon_pjrt.so")
    except Exception:
        hook = None
    mod = types.ModuleType("antenv.axon_hooks")
    mod.get_axon_ntff_profile_hook = lambda: hook
    mod.set_axon_ntff_profile_hook = lambda h: None
    sys.modules["antenv.axon_hooks"] = mod


def _fold_rope(w: np.ndarray, n_heads: int) -> np.ndarray:
    """Return W' with the (sin==cos) RoPE mixing folded in: x@W' = M(x@W) per head."""
    wf = w.reshape(D, n_heads, HD)
    lo, hi = wf[..., : HD // 2], wf[..., HD // 2 :]
    return np.concatenate([lo - hi, hi + lo], axis=-1).reshape(D, n_heads * HD)


def _cos_table() -> np.ndarray:
    inv_freq = 1.0 / (ROPE_BASE ** (np.arange(0, HD, 2, dtype=np.float64) / HD))
    freqs = np.arange(N, dtype=np.float64)[:, None] * inv_freq[None, :]  # [N, 64]
    emb = np.concatenate([freqs, freqs], axis=-1)  # [N, 128]
    return np.cos(emb).T.astype(np.float32).copy()  # [128, N]


_NC_CACHE: dict = {}


def _build_nc():
    if "nc" in _NC_CACHE:
        return _NC_CACHE["nc"]

    import concourse.bacc as bacc
    import concourse.mybir as mybir
    import concourse.tile as tile
    from concourse.bass import ts
    from concourse.masks import make_identity

    f32 = mybir.dt.float32
    bf16 = mybir.dt.bfloat16
    AFT = mybir.ActivationFunctionType
    KD = D // 128  # 16 contraction chunks
    NT = N // 128  # 16 m tiles of 128
    NC512 = N // 512  # 4 chunks of 512
    DC = DSLICE // 128  # 4 output-column chunks of 128

    nc = bacc.Bacc(target_bir_lowering=False, debug=False, num_devices=N_CORES)

    xt = nc.dram_tensor("xt", [D, N], bf16, kind="ExternalInput")  # x[b].T
    wq = nc.dram_tensor("wq", [D, JL], bf16, kind="ExternalInput")  # folded+scaled
    wk = nc.dram_tensor("wk", [D, HD], bf16, kind="ExternalInput")  # folded
    wv = nc.dram_tensor("wv", [D, HD], bf16, kind="ExternalInput")
    wo = nc.dram_tensor("wo", [H * HD, DSLICE], bf16, kind="ExternalInput")
    cost = nc.dram_tensor("cost", [HD, N], f32, kind="ExternalInput")
    # transposed output: outT[d, n]; host transposes back
    out = nc.dram_tensor("out", [DSLICE, N], f32, kind="ExternalOutput")

    xt_v = xt.rearrange("(ko p) n -> p ko n", p=128)
    wq_v = wq.rearrange("(ko p) j -> p ko j", p=128)
    wk_v = wk.rearrange("(ko p) j -> p ko j", p=128)
    wv_v = wv.rearrange("(ko p) j -> p ko j", p=128)
    wo_v = wo.rearrange("(ko p) d -> p ko d", p=128)

    from concourse.tile import add_dep_helper

    with tile.TileContext(nc) as tc:
        with (
            tc.tile_pool(name="big", bufs=1) as big_pool,
            tc.tile_pool(name="ag", bufs=3) as ag_pool,
            tc.tile_pool(name="otn", bufs=6) as otn_pool,
            tc.tile_pool(name="wpool", bufs=1) as w_pool,
            tc.tile_pool(name="work", bufs=1) as work_pool,
            tc.tile_pool(name="st", bufs=6) as st_pool,
            tc.tile_pool(name="ev", bufs=2) as ev_pool,
            tc.tile_pool(name="psum", bufs=2, space="PSUM") as ps_pool,
            tc.tile_pool(name="psacc", bufs=2, space="PSUM") as psacc_pool,
            tc.tile_pool(name="dram", bufs=1, space="DRAM") as dram_pool,
        ):
            # ---- persistent SBUF tensors ----
            x_sb = big_pool.tile([128, KD, N], bf16, tag="big")
            wq_sb = w_pool.tile([128, KD, JL], bf16, tag="wq")
            wk_sb = w_pool.tile([128, KD, HD], bf16, tag="wk")
            wv_sb = w_pool.tile([128, KD, HD], bf16, tag="wv")
            wo_sb = w_pool.tile([128, KD, DSLICE], bf16, tag="wo")
            cos_sb = w_pool.tile([128, N], f32, tag="cos")
            qT_sb = work_pool.tile([128, G, N], bf16, tag="qT")
            kT_sb = work_pool.tile([128, N], bf16, tag="kT")
            vT_sb = ag_pool.tile([128, HKV, N], bf16, tag="agsb", name="vT_sb")[:, 0, :]
            v_sb = work_pool.tile([128, N], bf16, tag="v")  # [m-part, mt*128+hd]
            ones_sb = work_pool.tile([128, 128], bf16, tag="ones")
            ident_sb = work_pool.tile([128, 128], bf16, tag="ident")

            nc.gpsimd.memset(ones_sb[:], 1.0)
            make_identity(nc, ident_sb[:])

            # ---- input DMAs (weights needed first come first) ----
            nc.sync.dma_start(wk_sb[:], wk_v[:])
            nc.sync.dma_start(wv_sb[:], wv_v[:])
            nc.sync.dma_start(cos_sb[:], cost[:, :])
            for kd in range(KD):
                nc.sync.dma_start(x_sb[:, kd, :], xt_v[:, kd, :])
            nc.sync.dma_start(wq_sb[:], wq_v[:])
            nc.sync.dma_start(wo_sb[:], wo_v[:])

            # ---- projections ----
            # k first (attention needs full kT before any head starts)
            for ncx in range(NC512):
                ps = ps_pool.tile([128, 512], f32, tag="mm")
                for kd in range(KD):
                    nc.tensor.matmul(
                        ps,
                        lhsT=wk_sb[:, kd, :],
                        rhs=x_sb[:, kd, ts(ncx, 512)],
                        start=(kd == 0),
                        stop=(kd == KD - 1),
                    )
                nc.vector.tensor_mul(kT_sb[:, ts(ncx, 512)], ps, cos_sb[:, ts(ncx, 512)])

            # v (as vT, then PE-transpose into natural [m, hd] layout)
            for ncx in range(NC512):
                ps = ps_pool.tile([128, 512], f32, tag="mm")
                for kd in range(KD):
                    nc.tensor.matmul(
                        ps,
                        lhsT=wv_sb[:, kd, :],
                        rhs=x_sb[:, kd, ts(ncx, 512)],
                        start=(kd == 0),
                        stop=(kd == KD - 1),
                    )
                nc.vector.tensor_copy(vT_sb[:, ts(ncx, 512)], ps)
            for q4 in range(NT // 4):
                ps_t = psacc_pool.tile([128, 512], bf16, tag="sums")
                for j in range(4):
                    mt = q4 * 4 + j
                    nc.tensor.transpose(
                        ps_t[:, ts(j, 128)], vT_sb[:, ts(mt, 128)], ident_sb[:]
                    )
                nc.vector.tensor_copy(v_sb[:, ts(q4, 512)], ps_t)

            # q (4 heads)
            for h in range(G):
                for ncx in range(NC512):
                    ps = ps_pool.tile([128, 512], f32, tag="mm")
                    for kd in range(KD):
                        nc.tensor.matmul(
                            ps,
                            lhsT=wq_sb[:, kd, ts(h, 128)],
                            rhs=x_sb[:, kd, ts(ncx, 512)],
                            start=(kd == 0),
                            stop=(kd == KD - 1),
                        )
                    nc.vector.tensor_mul(
                        qT_sb[:, h, ts(ncx, 512)], ps, cos_sb[:, ts(ncx, 512)]
                    )

            # outT fp32 accumulator for the output projection (reuses x_sb's slot
            # footprint only after x is dead; separate tag keeps sizes honest)
            outT_acc = big_pool.tile([128, DC, N], f32, tag="big")

            # ---- attention + per-head chunked AllGather + Wo slab accumulation ----
            ag_ins = []
            ag_outs = []
            for h in range(G):
                ag_ins.append(
                    dram_pool.tile([HD, N], bf16, tag=f"agi{h}", name=f"agi{h}")
                )
                ag_outs.append(
                    dram_pool.tile(
                        [HKV * HD, N], bf16, tag=f"ago{h}", name=f"ago{h}"
                    )
                )

            # leading tiny collective: absorbs cross-core rendezvous skew while
            # the PE is busy with projections, so the first real gather is cheap
            bar_in = dram_pool.tile([1, 128], bf16, tag="bar_in", name="bar_in")
            bar_out = dram_pool.tile([4, 128], bf16, tag="bar_out", name="bar_out")
            nc.gpsimd.collective_compute(
                "AllGather",
                mybir.AluOpType.bypass,
                replica_groups=[[0, 1, 2, 3], [4, 5, 6, 7]],
                ins=[bar_in[:].opt()],
                outs=[bar_out[:].opt()],
            )

            attn_last = {}

            def slab_contribution(h):
                """Add gathered head-h slab's term to the outT accumulator."""
                gate = attn_last.get(min(h + 1, G - 1))
                ag_v = ag_outs[h].rearrange("(r p) n -> p r n", p=128)
                ag_sb = ag_pool.tile([128, HKV, N], bf16, tag="agsb", name=f"agsb{h}")
                ag_dma = nc.sync.dma_start(ag_sb[:], ag_v[:])
                if gate is not None:
                    add_dep_helper(
                        ag_dma.ins,
                        gate.ins,
                        sync=True,
                        reason="keep slab DMA behind later otn DMAs",
                    )
                for dc in range(DC):
                    for nn in range(NC512):
                        ps = ps_pool.tile([128, 512], f32, tag="mm", name=f"sd{h}")
                        for r in range(HKV):
                            jc = r * G + h
                            mm = nc.tensor.matmul(
                                ps,
                                lhsT=wo_sb[:, jc, ts(dc, 128)],
                                rhs=ag_sb[:, r, ts(nn, 512)],
                                start=(r == 0),
                                stop=(r == HKV - 1),
                            )
                            if gate is not None:
                                # placement hint: keep slab matmuls out of the
                                # PE stream until head h+1's attention is done,
                                # so the PE never waits on an in-flight gather
                                add_dep_helper(
                                    mm.ins,
                                    gate.ins,
                                    sync=True,
                                    reason="slab after next head's attention",
                                )
                        if h == 0:
                            nc.vector.tensor_copy(outT_acc[:, dc, ts(nn, 512)], ps)
                        else:
                            nc.vector.tensor_add(
                                outT_acc[:, dc, ts(nn, 512)],
                                ps,
                                outT_acc[:, dc, ts(nn, 512)],
                            )

            for h in range(G):
                for half in range(NC512 // 2):
                    ncA, ncB = 2 * half, 2 * half + 1
                    otA = psacc_pool.tile([128, 512], f32, tag="ot", name="otA")
                    otB = psacc_pool.tile([128, 512], f32, tag="ot", name="otB")
                    sumsA = psacc_pool.tile([128, 512], f32, tag="sums", name="sumsA")
                    sumsB = psacc_pool.tile([128, 512], f32, tag="sums", name="sumsB")
                    for mt in range(NT):
                        sp = ps_pool.tile([128, 1024], f32, tag="mm", name="sp")
                        nc.tensor.matmul(
                            sp[:, :512],
                            lhsT=kT_sb[:, ts(mt, 128)],
                            rhs=qT_sb[:, h, ts(ncA, 512)],
                            start=True,
                            stop=True,
                        )
                        nc.tensor.matmul(
                            sp[:, 512:],
                            lhsT=kT_sb[:, ts(mt, 128)],
                            rhs=qT_sb[:, h, ts(ncB, 512)],
                            start=True,
                            stop=True,
                        )
                        st_sb = st_pool.tile([128, 1024], bf16, tag="st")
                        nc.scalar.activation(st_sb[:], sp[:], AFT.Exp)
                        nc.tensor.matmul(
                            otA,
                            lhsT=v_sb[:, ts(mt, 128)],
                            rhs=st_sb[:, :512],
                            start=(mt == 0),
                            stop=(mt == NT - 1),
                        )
                        nc.tensor.matmul(
                            otB,
                            lhsT=v_sb[:, ts(mt, 128)],
                            rhs=st_sb[:, 512:],
                            start=(mt == 0),
                            stop=(mt == NT - 1),
                        )
                        nc.tensor.matmul(
                            sumsA,
                            lhsT=ones_sb[:],
                            rhs=st_sb[:, :512],
                            start=(mt == 0),
                            stop=(mt == NT - 1),
                        )
                        sums_mm = nc.tensor.matmul(
                            sumsB,
                            lhsT=ones_sb[:],
                            rhs=st_sb[:, 512:],
                            start=(mt == 0),
                            stop=(mt == NT - 1),
                        )
                        attn_last[h] = sums_mm
                    for ncx, ot_ps, sums_ps in ((ncA, otA, sumsA), (ncB, otB, sumsB)):
                        recip_sb = ev_pool.tile([128, 512], f32, tag="recip")
                        nc.vector.reciprocal_approx_fast(recip_sb[:], sums_ps)
                        otn_sb = otn_pool.tile([128, 512], bf16, tag="otn")
                        nc.vector.tensor_mul(otn_sb[:], ot_ps, recip_sb[:])
                        nc.sync.dma_start(ag_ins[h][:, ts(ncx, 512)], otn_sb[:])

                # gather this head's slab across the 4 cores of this batch
                nc.gpsimd.collective_compute(
                    "AllGather",
                    mybir.AluOpType.bypass,
                    replica_groups=[[0, 1, 2, 3], [4, 5, 6, 7]],
                    ins=[ag_ins[h][:].opt()],
                    outs=[ag_outs[h][:].opt()],
                )
            # all slab matmuls after all attention: collectives 0-2 hide under
            # attention, collective 3 hides under slabs 0-2; only slab 3 is tail
            for h in range(G):
                slab_contribution(h)

            # ---- write transposed output ----
            for dc in range(DC):
                nc.sync.dma_start(out[ts(dc, 128), :], outT_acc[:, dc, :])

    nc.compile()
    _NC_CACHE["nc"] = nc
    return nc


def kernel(x, Wq, Wk, Wv, Wo):
    _install_axon_ntff_hook()
    import ml_dtypes

    import concourse.bass_utils as bass_utils

    bass_utils.upload_artifacts = lambda tmpdir: str(tmpdir)
    from concourse.bass_utils import run_bass_kernel_spmd

    x = np.asarray(x, dtype=np.float32)
    Wq = np.asarray(Wq, dtype=np.float32)
    Wk = np.asarray(Wk, dtype=np.float32)
    Wv = np.asarray(Wv, dtype=np.float32)
    Wo = np.asarray(Wo, dtype=np.float32)

    bf = ml_dtypes.bfloat16
    scale = np.float32(HD**-0.5)
    wq_f = (_fold_rope(Wq, H) * scale).astype(bf)  # [D, 2048]
    wk_f = _fold_rope(Wk, HKV).astype(bf)  # [D, 512]
    wv_f = Wv.astype(bf)  # [D, 512]
    wo_f = Wo.astype(bf)  # [2048, D]
    cos_t = _cos_table()  # [128, N] fp32

    xt = [np.ascontiguousarray(x[b].T).astype(bf) for b in range(B)]

    in_maps = []
    for c in range(N_CORES):
        b, g = divmod(c, HKV)
        in_maps.append(
            {
                "xt": xt[b],
                "wq": np.ascontiguousarray(wq_f[:, g * JL : (g + 1) * JL]),
                "wk": np.ascontiguousarray(wk_f[:, g * HD : (g + 1) * HD]),
                "wv": np.ascontiguousarray(wv_f[:, g * HD : (g + 1) * HD]),
                "wo": np.ascontiguousarray(wo_f[:, g * DSLICE : (g + 1) * DSLICE]),
                "cost": cos_t,
            }
        )

    nc = _build_nc()
    res = run_bass_kernel_spmd(nc, in_maps, list(range(N_CORES)))

    out = np.empty((B, N, D), dtype=np.float32)
    for c in range(N_CORES):
        b, g = divmod(c, HKV)
        out[b, :, g * DSLICE : (g + 1) * DSLICE] = res.results[c]["out"].T
    return out
